# revision 23
# baseline (speedup 1.0000x reference)
"""Trainium2 Bass kernel for nn_DeconvDft2dLayer.

y = irfft2(gmf * rfft2(pad(x)))  with x (64,512,512), w (3,3), y (64,768,768).

Strategy: data-parallel over batch (8 samples per NeuronCore). Per sample the
FFTs are evaluated as DFT matmuls on the tensor engine (fp32r, full rate):

  A : S1^T[w,k] = sum_h x[h,w] W2[h,k]            k in [0,385)   (fft-H, halved
      via Hermitian symmetry of the real input)
  B1: S[k,j]    = sum_w S1[k,w] C1[w,j]           k in [0,384)
  B2: S'[k',j]  = sum_w S1[k',w] conj(C1)[w,j]    k' in [1,385)
  C : T[r,j] (768 rows) = gmf[rho(r)] * X,  X = S rows (r<384) or conj(S') rows
      (rho(r) = r for r<384, 1151-r otherwise) -- elementwise on VectorE
  D : U^T[j,n]  = sum_r T[r,j] Atil[r,n],  Atil[r,n] = e^{2i pi n rho(r)/768}/768^2
      (Karatsuba: 3 real matmuls via Tsum = Tre+Tim against Are+Aim)
  E : y[n,m]    = sum_j Ure[j,n] Bre[j,m] + Uim[j,n] Bimn[j,m]
      Bre = w_j cos(2 pi j m/768), Bimn = -w_j sin(2 pi j m/768)
      (Nyquist j=384: Bimn[384]=0 and Bimn[0]=0, so Ure[384] rides the dead
      (imag, j=0) weight row with Bre[384]=(-1)^m planted in bmat; both
      m-halves interleaved so the PE never drains at a psum-group boundary)

gmf and the DFT matrices are tiny 3x3-derived constants computed host-side
(float64) and replicated to all cores; no cross-device communication.
All DRAM tensors are host-packed in the exact SBUF tile layout so every DMA is
128 large contiguous descriptors; c1 is DMA'd per-component so stage A can
start as soon as x and its first 0.8 MB component land.
"""
import os

import numpy as np
import ml_dtypes

import concourse.bacc as bacc
import concourse.mybir as mybir
import concourse.tile as tile
from concourse.bass_utils import run_bass_kernel_spmd

F32 = mybir.dt.float32
F32R = mybir.dt.float32r
BF16 = mybir.dt.bfloat16

HP = 768          # padded grid
J = 385           # rfft half length (768//2+1)
JP = 386          # padded to even for fp32r free-dim constraint
NS = 8            # samples per core
NCORES = 8

LAST_EXEC_NS = None
LAST_RESULTS = None


def _build_constants(w):
    """Host-side constants (float64 -> float32/bf16), packed in SBUF layout."""
    w = np.asarray(w, np.float64)
    hm1 = np.zeros((HP, HP)); hm1[:3, :3] = w
    gm1f = 1.0 / np.fft.rfft2(hm1)
    gm2f = np.roll(gm1f[::-1, :], shift=1, axis=0)
    gm3f = np.roll(gm1f[:, ::-1], shift=1, axis=1)
    gm4f = np.roll(gm3f[::-1, :], shift=1, axis=0)
    gmf = (gm1f * gm2f) * (gm3f * gm4f)          # (768, 385) complex

    h = np.arange(512)
    k = np.arange(J)
    ph = np.exp(-2j * np.pi * (np.outer(h + 128, k) % HP) / HP)   # (512,385)
    j = np.arange(J)
    c1 = np.zeros((3, 512, JP))
    c1[0, :, :J] = ph.real            # C1 == W2 (same 512x385 phase table)
    c1[1, :, :J] = ph.imag
    c1[2, :, :J] = -ph.imag

    r = np.arange(HP)
    rho = np.where(r < 384, r, 1151 - r)
    pq = np.zeros((2, HP, JP))
    pq[0, :, :J] = gmf.real[rho, :]
    pq[1, :, :J] = gmf.imag[rho, :]

    n = np.arange(HP)
    pa = np.exp(2j * np.pi * (np.outer(rho, n) % HP) / HP) / (HP * HP)
    atil = np.stack([pa.real, pa.imag, pa.real + pa.imag])   # (3, 768, 768)

    # Nyquist-column fold: T[.,384] is (near-)Hermitian in kh because the
    # operator maps real->real, so Ure[384] = sum over rho 0..384 with
    # doubled weights instead of all 768 rows. Rows rho 384..415 (tmat
    # chunk 5, partitions 96..127) are kept explicit at weight 1 -- their
    # mirror partners rho 353..383 drop to weight 1 -- via a 4th const
    # chunk that is zero on partitions 0..95, so every matmul stays K=128.
    wgt = np.full(384, 2.0); wgt[0] = 1.0; wgt[353:384] = 1.0
    any_arr = np.zeros((2, 4, 128, HP))
    for rc in range(3):
        rows = np.arange(rc * 128, rc * 128 + 128)
        any_arr[0, rc] = wgt[rows, None] * pa.real[rows]
        any_arr[1, rc] = wgt[rows, None] * pa.imag[rows]
    any_arr[0, 3, 96:128] = pa.real[736:768]
    any_arr[1, 3, 96:128] = pa.imag[736:768]

    m = np.arange(HP)
    wj = np.where((j == 0) | (j == 384), 1.0, 2.0)
    ang = 2 * np.pi * (np.outer(j, m) % HP) / HP
    bre = wj[:, None] * np.cos(ang)              # (385, 768)
    bimn = -wj[:, None] * np.sin(ang)
    bmat = np.stack([bre[:384], bimn[:384]])     # (2, 384, 768)
    # dead-row Nyquist fold: bimn[0] == 0, so the (imag, j=0) weight row is
    # free -- plant Bre[384] = (-1)^m there; stage E picks up the Nyquist
    # term from ut row (comp1, jc0, p0) := Ure[384] with no extra matmuls
    bmat[1, 0, :] = bre[384]

    f = np.float32
    c1p = np.ascontiguousarray(c1.reshape(3, 4, 128, JP).transpose(2, 0, 1, 3), f)
    return {
        # packed to SBUF layouts: leading dim = partition
        "c1a": np.ascontiguousarray(c1p[:, 0:1]).astype(ml_dtypes.bfloat16),   # re   (stage A comp0)
        "c1b": np.ascontiguousarray(c1p[:, 1:2]).astype(ml_dtypes.bfloat16),   # im   (stage A comp1)
        "c1c": np.ascontiguousarray(c1p[:, 2:3]).astype(ml_dtypes.bfloat16),   # -im  (stage B only)
        "pq": np.ascontiguousarray(
            pq.reshape(2, 6, 128, JP).transpose(2, 0, 1, 3)).astype(
                ml_dtypes.bfloat16),
        "atil": np.ascontiguousarray(atil.reshape(3, 6, 128, HP).transpose(2, 0, 1, 3)).astype(ml_dtypes.bfloat16),
        "any": np.ascontiguousarray(any_arr.transpose(2, 0, 1, 3)).astype(ml_dtypes.bfloat16),
        "bmat": np.ascontiguousarray(bmat.reshape(2, 3, 128, HP).transpose(2, 0, 1, 3)).astype(ml_dtypes.bfloat16),
    }


def _build_program(ns=NS):
    nc = bacc.Bacc("TRN2", target_bir_lowering=False, debug=False,
                   num_devices=NCORES)
    x_ext = nc.declare_dram_parameter("x", [ns, 128, 4, 512], BF16, isOutput=False)
    y_ext = nc.declare_dram_parameter("y", [ns, 128, 6, HP], F32, isOutput=True)
    c1a_ext = nc.declare_dram_parameter("c1a", [128, 1, 4, JP], BF16, isOutput=False)
    c1b_ext = nc.declare_dram_parameter("c1b", [128, 1, 4, JP], BF16, isOutput=False)
    c1c_ext = nc.declare_dram_parameter("c1c", [128, 1, 4, JP], BF16, isOutput=False)
    pq_ext = nc.declare_dram_parameter("pq", [128, 2, 6, JP], BF16, isOutput=False)
    atil_ext = nc.declare_dram_parameter("atil", [128, 3, 6, HP], BF16, isOutput=False)
    any_ext = nc.declare_dram_parameter("any", [128, 2, 4, HP], BF16, isOutput=False)
    bmat_ext = nc.declare_dram_parameter("bmat", [128, 2, 3, HP], BF16, isOutput=False)

    MUL = mybir.AluOpType.mult
    ADD = mybir.AluOpType.add
    SUB = mybir.AluOpType.subtract

    # tmat component order
    TRE, TIM, TSUM = 0, 1, 2

    with tile.TileContext(nc) as tc:
        with tc.tile_pool(name="const", bufs=1) as cpool, \
             tc.tile_pool(name="data", bufs=1) as dpool, \
             tc.tile_pool(name="xin", bufs=1) as xpool, \
             tc.tile_pool(name="yout", bufs=2) as ypool, \
             tc.tile_pool(name="scr", bufs=2) as spool, \
             tc.tile_pool(name="psum", bufs=8, space="PSUM") as ppool:

            # sample-0 input + stage-A consts land per h-chunk, interleaved,
            # so A's first matmul issues after ~0.5 MB instead of ~1.8 MB
            # (the first psum group then paces with the DMA stream)
            xts = []
            xt0 = xpool.tile([128, 4, 512], BF16, tag="x")
            c1_t = cpool.tile([128, 3, 4, JP], BF16, tag="c1")
            for hc in range(4):
                nc.sync.dma_start(out=c1_t[:, 0, hc, :], in_=c1a_ext[:, 0, hc])
                nc.sync.dma_start(out=xt0[:, hc], in_=x_ext[0, :, hc])
            xts.append(xt0)
            nc.sync.dma_start(out=c1_t[:, 1:2], in_=c1b_ext[:])
            nc.sync.dma_start(out=c1_t[:, 2:3], in_=c1c_ext[:])
            pq_t = cpool.tile([128, 2, 6, JP], BF16, tag="pq")
            nc.sync.dma_start(out=pq_t[:], in_=pq_ext[:])
            a_t = cpool.tile([128, 3, 6, HP], BF16, tag="atil")
            nc.sync.dma_start(out=a_t[:], in_=atil_ext[:])
            any_t = cpool.tile([128, 2, 4, HP], BF16, tag="any")
            nc.sync.dma_start(out=any_t[:], in_=any_ext[:])
            b_t = cpool.tile([128, 2, 3, HP], BF16, tag="bmat")
            nc.sync.dma_start(out=b_t[:], in_=bmat_ext[:])

            def mm(ps, lhsT, rhs, start, stop):
                nc.tensor.matmul(ps, lhsT=lhsT, rhs=rhs, start=start, stop=stop)

            # PE warm-up: ~12 bf16 matmuls on zeroed SBUF keep the PE busy
            # (and the clock ramping) while the x/c1 DMAs stream in; sized
            # to drain just before the stage-A inputs land
            wz = cpool.tile([128, 512], BF16, tag="warm")
            nc.vector.memset(wz[:], 0.0)
            pw = ppool.tile([128, 512], F32, tag="ps")
            for _ in range(12):
                mm(pw[:], wz[:, 0:128], wz[:], True, True)

            for b in range(ns):
                xt = xts[b]
                if b + 1 < ns:   # prefetch next sample
                    nxt = xpool.tile([128, 4, 512], BF16, tag="x")
                    nc.sync.dma_start(out=nxt[:], in_=x_ext[b + 1])
                    xts.append(nxt)

                s1 = dpool.tile([128, 2, 4, JP], BF16, tag="s1")
                tmat = dpool.tile([128, 3, 6, JP], BF16, tag="tmat")
                tny = dpool.tile([128, 6], BF16, tag="tny")
                ut = dpool.tile([128, 2, 3, HP], BF16, tag="ut")

                # ---- stage A ----
                for comp in range(2):
                    for wc in range(4):
                        ps = ppool.tile([128, JP], F32, tag="ps")
                        for hc in range(4):
                            mm(ps[:], xt[:, hc, wc * 128:(wc + 1) * 128],
                               c1_t[:, comp, hc, :], hc == 0, hc == 3)
                        nc.scalar.copy(s1[:, comp, wc, :], ps[:])

                # ---- stages B + C, 6 chunks of T ----
                for c in range(6):
                    mirror = c >= 3
                    lo = (c - 3) * 128 + 1 if mirror else c * 128
                    ksl = slice(lo, lo + 128)
                    ps_re = ppool.tile([128, JP], F32, tag="ps")
                    ps_im = ppool.tile([128, JP], F32, tag="ps")
                    # real part of S (or S')
                    for wc in range(4):
                        mm(ps_re[:], s1[:, 0, wc, ksl], c1_t[:, 0, wc, :], wc == 0, False)
                    for wc in range(4):
                        # S: + S1im @ (-C1im) ; S': + S1im @ (+C1im)
                        mm(ps_re[:], s1[:, 1, wc, ksl],
                           c1_t[:, 1 if mirror else 2, wc, :], False, wc == 3)
                    # imag part
                    for wc in range(4):
                        # S: + S1re @ C1im ; S': + S1re @ (-C1im)
                        mm(ps_im[:], s1[:, 0, wc, ksl],
                           c1_t[:, 2 if mirror else 1, wc, :], wc == 0, False)
                    for wc in range(4):
                        mm(ps_im[:], s1[:, 1, wc, ksl], c1_t[:, 0, wc, :], False, wc == 3)

                    # ---- stage C on this chunk ----
                    t_re = tmat[:, TRE, c, :]
                    t_im = tmat[:, TIM, c, :]
                    scr = spool.tile([128, JP], BF16, tag="scr")
                    scr2 = spool.tile([128, JP], BF16, tag="scr")
                    nc.vector.tensor_tensor(out=t_re, in0=pq_t[:, 0, c, :], in1=ps_re[:], op=MUL)
                    nc.vector.tensor_tensor(out=scr[:], in0=pq_t[:, 1, c, :], in1=ps_im[:], op=MUL)
                    nc.vector.tensor_tensor(out=t_re, in0=t_re, in1=scr[:],
                                            op=ADD if mirror else SUB)
                    nc.vector.tensor_tensor(out=t_im, in0=pq_t[:, 1, c, :], in1=ps_re[:], op=MUL)
                    nc.vector.tensor_tensor(out=scr2[:], in0=pq_t[:, 0, c, :], in1=ps_im[:], op=MUL)
                    nc.vector.tensor_tensor(out=t_im, in0=t_im, in1=scr2[:],
                                            op=SUB if mirror else ADD)
                    # only the j=384 column of -Tim is ever used (Nyquist)
                    nc.vector.tensor_scalar_mul(tny[:, c:c + 1],
                                                tmat[:, TIM, c, 384:385], -1.0)
                    nc.vector.tensor_tensor(out=tmat[:, TSUM, c, :], in0=t_re,
                                            in1=t_im, op=ADD)

                # ---- stage D, full j-chunks (Karatsuba: 18 matmuls/group) ----
                for jc in range(3):
                    jsl = slice(jc * 128, jc * 128 + 128)
                    for nh in range(2):
                        nsl = slice(nh * 384, nh * 384 + 384)
                        pm1 = ppool.tile([128, 384], F32, tag="ps")
                        pm2 = ppool.tile([128, 384], F32, tag="ps")
                        pm3 = ppool.tile([128, 384], F32, tag="ps")
                        for rc in range(6):
                            mm(pm1[:], tmat[:, TRE, rc, jsl], a_t[:, 0, rc, nsl], rc == 0, rc == 5)
                        for rc in range(6):
                            mm(pm2[:], tmat[:, TIM, rc, jsl], a_t[:, 1, rc, nsl], rc == 0, rc == 5)
                        for rc in range(6):
                            mm(pm3[:], tmat[:, TSUM, rc, jsl], a_t[:, 2, rc, nsl], rc == 0, rc == 5)
                        # Ure = M1 - M2 ; Uim = M3 - M1 - M2
                        scrd = spool.tile([128, JP], F32, tag="scr")
                        nc.scalar.copy(scrd[:, :384], pm2[:, :])
                        nc.vector.tensor_tensor(out=ut[:, 0, jc, nsl], in0=pm1[:],
                                                in1=scrd[:, :384], op=SUB)
                        nc.vector.tensor_tensor(out=ut[:, 1, jc, nsl], in0=pm3[:],
                                                in1=ut[:, 0, jc, nsl], op=SUB)
                        nc.vector.scalar_tensor_tensor(
                            out=ut[:, 1, jc, nsl], in0=scrd[:, :384], scalar=-2.0,
                            in1=ut[:, 1, jc, nsl], op0=MUL, op1=ADD)

                # ---- stage D, Nyquist column j=384: Ure only (Uim is killed
                # by bimn[384] == 0), landed directly in the dead ut row ----
                for nh in range(2):
                    nsl = slice(nh * 384, nh * 384 + 384)
                    ps_u = ppool.tile([1, 384], F32, tag="ps")
                    for rc in range(3):
                        # += Tre @ (w * Are), rho 0..383
                        mm(ps_u[:], tmat[:, TRE, rc, 384:385], any_t[:, 0, rc, nsl],
                           rc == 0, False)
                    for rc in range(3):
                        # += (-Tim) @ (w * Aim)
                        mm(ps_u[:], tny[:, rc:rc + 1], any_t[:, 1, rc, nsl],
                           False, False)
                    # rho 384..415 explicit (chunk-5 rows; const zero on p<96)
                    mm(ps_u[:], tmat[:, TRE, 5, 384:385], any_t[:, 0, 3, nsl],
                       False, False)
                    mm(ps_u[:], tny[:, 5:6], any_t[:, 1, 3, nsl],
                       False, True)
                    nc.scalar.copy(ut[0:1, 1, 0, nsl], ps_u[:, :])

                # ---- stage E: both m-halves interleaved so the PE never
                # drains at a psum-group boundary ----
                for nch in range(6):
                    nsl = slice(nch * 128, nch * 128 + 128)
                    ytc = ypool.tile([128, HP], F32, tag="y")
                    ps0 = ppool.tile([128, 384], F32, tag="ps")
                    ps1 = ppool.tile([128, 384], F32, tag="ps")
                    for comp in range(2):
                        for jc in range(3):
                            first = comp == 0 and jc == 0
                            last = comp == 1 and jc == 2
                            lhsT = ut[:, comp, jc, nsl]
                            mm(ps0[:], lhsT, b_t[:, comp, jc, 0:384], first, last)
                            mm(ps1[:], lhsT, b_t[:, comp, jc, 384:768], first, last)
                    # copies split across Scalar/Vector so both m-halves
                    # land in parallel (shorter tail on the final group)
                    nc.scalar.copy(ytc[:, 0:384], ps0[:])
                    nc.sync.dma_start(out=y_ext[b, :, nch, 0:384], in_=ytc[:, 0:384])
                    nc.vector.tensor_copy(out=ytc[:, 384:768], in_=ps1[:])
                    nc.sync.dma_start(out=y_ext[b, :, nch, 384:768], in_=ytc[:, 384:768])

    nc.compile()
    return nc


_PROGRAM_CACHE = {}


def kernel(x, w, trace=False):
    global LAST_EXEC_NS, LAST_RESULTS
    x = np.asarray(x, np.float32)
    B = x.shape[0]
    # pack to the SBUF tile layout: x_dev[b, p, c, w] = x[b, c*128+p, w]
    x_dev = np.ascontiguousarray(
        x.reshape(B, 4, 128, 512).transpose(0, 2, 1, 3)).astype(ml_dtypes.bfloat16)
    consts = _build_constants(w)
    if NS not in _PROGRAM_CACHE:
        _PROGRAM_CACHE[NS] = _build_program(NS)
    nc = _PROGRAM_CACHE[NS]
    in_maps = []
    for core in range(NCORES):
        m = {"x": x_dev[core * NS:(core + 1) * NS]}
        m.update(consts)
        in_maps.append(m)
    if trace:
        os.environ.pop("BASS_NEVER_TRACE", None)
        res = run_bass_kernel_spmd(nc, in_maps, list(range(NCORES)), trace=True)
    else:
        # profiling needs the antenv NTFF shim; never let a stray BASS_TRACE
        # env var route us down that path during plain runs
        os.environ["BASS_NEVER_TRACE"] = "1"
        try:
            res = run_bass_kernel_spmd(nc, in_maps, list(range(NCORES)), trace=False)
        finally:
            os.environ.pop("BASS_NEVER_TRACE", None)
    LAST_EXEC_NS = res.exec_time_ns
    LAST_RESULTS = res
    # unshard: y_dev[b, p, c, m] -> y[b, c*128+p, m]
    y_dev = np.concatenate([res.results[i]["y"] for i in range(NCORES)], axis=0)
    y = y_dev.transpose(0, 2, 1, 3).reshape(B, HP, HP)
    return np.ascontiguousarray(y, np.float32)
